# revision 1
# baseline (speedup 1.0000x reference)
"""Trainium2 Bass kernel for nn_ContrastiveLoss (wav2vec2-style contrastive loss).

Shapes (hardcoded): B=8, C=256, T=1024, M=512 masked positions, K=100 negatives.
Sharding: pure data parallel — batch row b -> NeuronCore b (8 cores).

Per core the dominant work is streaming negatives[b] ([M, K, C] f32, 52.4 MB)
from HBM once, computing per (m, k):
    dot[m,k]   = sum_c neg[m,k,c] * ctx_m[m,c]      (VectorE scalar_tensor_tensor
                                                     fused multiply + accumulate)
    sumsq[m,k] = sum_c neg[m,k,c]^2                 (ScalarE activation Square
                                                     with accum_out)
then cosine normalization, logsumexp over K+1 logits, and per-row loss.
The device returns per-row losses [128, 4] per core; the host sums and divides.
"""

import numpy as np

TEMP = 0.1
EPS = 1e-8
B, C, T = 8, 256, 1024
M = 512  # masked positions per batch row
K = 100  # negatives per masked position
P = 128  # partitions
G = M // P  # m-groups per core (4)
KCH = 10  # k's per streamed tile: [128, KCH, C] f32 = 1.25 MB
NKC = K // KCH  # stream tiles per m-group (10)

_NC = None


def _build_nc():
    import concourse.bacc as bacc
    import concourse.tile as tile
    from concourse import mybir

    f32 = mybir.dt.float32
    Alu = mybir.AluOpType
    Act = mybir.ActivationFunctionType

    nc = bacc.Bacc(trn_type="TRN2")
    neg = nc.dram_tensor("neg", [M, K, C], f32, kind="ExternalInput")
    ctxg = nc.dram_tensor("ctxg", [M, C], f32, kind="ExternalInput")
    posg = nc.dram_tensor("posg", [M, C], f32, kind="ExternalInput")
    rowloss = nc.dram_tensor("rowloss", [P, G], f32, kind="ExternalOutput")

    with tile.TileContext(nc) as tc:
        with (
            tc.tile_pool(name="stream", bufs=4) as stream,
            tc.tile_pool(name="grp", bufs=2) as grp,
            tc.tile_pool(name="scr", bufs=2) as scrp,
            tc.tile_pool(name="outp", bufs=1) as outp,
        ):
            out_t = outp.tile([P, G], f32)
            for g in range(G):
                m0 = g * P
                ctx_t = grp.tile([P, C], f32, tag="ctx")
                pos_t = grp.tile([P, C], f32, tag="pos")
                nc.sync.dma_start(out=ctx_t[:], in_=ctxg[m0 : m0 + P, :])
                nc.sync.dma_start(out=pos_t[:], in_=posg[m0 : m0 + P, :])

                css = grp.tile([P, 1], f32, tag="css")
                pss = grp.tile([P, 1], f32, tag="pss")
                cpd = grp.tile([P, 1], f32, tag="cpd")
                scr = scrp.tile([P, C], f32, tag="scr")
                nc.vector.scalar_tensor_tensor(
                    out=scr[:], in0=ctx_t[:], scalar=1.0, in1=ctx_t[:],
                    op0=Alu.mult, op1=Alu.mult, accum_out=css[:],
                )
                nc.vector.scalar_tensor_tensor(
                    out=scr[:], in0=pos_t[:], scalar=1.0, in1=pos_t[:],
                    op0=Alu.mult, op1=Alu.mult, accum_out=pss[:],
                )
                nc.vector.scalar_tensor_tensor(
                    out=scr[:], in0=ctx_t[:], scalar=1.0, in1=pos_t[:],
                    op0=Alu.mult, op1=Alu.mult, accum_out=cpd[:],
                )
                # 1/max(||ctx||, EPS), 1/max(||pos||, EPS)
                crn = grp.tile([P, 1], f32, tag="crn")
                prn = grp.tile([P, 1], f32, tag="prn")
                nc.scalar.sqrt(css[:], css[:])
                nc.scalar.sqrt(pss[:], pss[:])
                nc.vector.tensor_scalar_max(css[:], css[:], EPS)
                nc.vector.tensor_scalar_max(pss[:], pss[:], EPS)
                nc.vector.reciprocal(crn[:], css[:])
                nc.vector.reciprocal(prn[:], pss[:])

                rawdots = grp.tile([P, K], f32, tag="rawdots")
                negss = grp.tile([P, K], f32, tag="negss")
                for t in range(NKC):
                    nt = stream.tile([P, KCH, C], f32, tag="nt")
                    nc.sync.dma_start(
                        out=nt[:],
                        in_=neg[m0 : m0 + P, t * KCH : (t + 1) * KCH, :],
                    )
                    for j in range(KCH):
                        k = t * KCH + j
                        nc.vector.scalar_tensor_tensor(
                            out=scr[:], in0=nt[:, j, :], scalar=1.0, in1=ctx_t[:],
                            op0=Alu.mult, op1=Alu.mult,
                            accum_out=rawdots[:, k : k + 1],
                        )
                        scr2 = scrp.tile([P, C], f32, tag="scr2")
                        nc.scalar.activation(
                            out=scr2[:], in_=nt[:, j, :], func=Act.Square,
                            accum_out=negss[:, k : k + 1],
                        )

                # normalize negatives: 1/max(sqrt(sumsq), EPS)
                nrn = grp.tile([P, K], f32, tag="nrn")
                nc.scalar.sqrt(negss[:], negss[:])
                nc.vector.tensor_scalar_max(negss[:], negss[:], EPS)
                nc.vector.reciprocal(nrn[:], negss[:])

                # logits (cosine sims): col 0 = positive, cols 1..K = negatives
                logits = grp.tile([P, K + 1], f32, tag="logits")
                nc.vector.scalar_tensor_tensor(
                    out=logits[:, 0:1], in0=cpd[:], scalar=crn[:], in1=prn[:],
                    op0=Alu.mult, op1=Alu.mult,
                )
                nc.vector.scalar_tensor_tensor(
                    out=logits[:, 1 : K + 1], in0=rawdots[:], scalar=crn[:],
                    in1=nrn[:], op0=Alu.mult, op1=Alu.mult,
                )

                # logsumexp over K+1 sims at temperature TEMP
                mx = grp.tile([P, 1], f32, tag="mx")
                mxs = grp.tile([P, 1], f32, tag="mxs")
                nc.vector.reduce_max(mx[:], logits[:], axis=mybir.AxisListType.X)
                nc.vector.tensor_scalar_mul(mxs[:], mx[:], -1.0 / TEMP)
                esc = scrp.tile([P, K + 1], f32, tag="esc")
                se = grp.tile([P, 1], f32, tag="se")
                nc.scalar.activation(
                    out=esc[:], in_=logits[:], func=Act.Exp,
                    scale=1.0 / TEMP, bias=mxs[:], accum_out=se[:],
                )
                lnse = grp.tile([P, 1], f32, tag="lnse")
                nc.scalar.activation(out=lnse[:], in_=se[:], func=Act.Ln)
                # rowloss = ln(se) + mx/TEMP - pos_sim/TEMP
                t1 = grp.tile([P, 1], f32, tag="t1")
                nc.vector.scalar_tensor_tensor(
                    out=t1[:], in0=mx[:], scalar=1.0 / TEMP, in1=lnse[:],
                    op0=Alu.mult, op1=Alu.add,
                )
                nc.vector.scalar_tensor_tensor(
                    out=out_t[:, g : g + 1], in0=logits[:, 0:1],
                    scalar=-1.0 / TEMP, in1=t1[:], op0=Alu.mult, op1=Alu.add,
                )
            nc.sync.dma_start(out=rowloss[:], in_=out_t[:])
    nc.finalize()
    return nc


def _get_nc():
    global _NC
    if _NC is None:
        _NC = _build_nc()
    return _NC


def kernel(context, positive, negatives, mask_indices, num_masked):
    from concourse.bass_utils import run_bass_kernel_spmd

    context = np.asarray(context, dtype=np.float32)
    positive = np.asarray(positive, dtype=np.float32)
    negatives = np.asarray(negatives, dtype=np.float32)
    mask = np.asarray(mask_indices).astype(bool)
    nm = int(np.asarray(num_masked))
    assert nm == M, f"kernel hardcodes num_masked={M}, got {nm}"
    assert context.shape == (B, C, T) and negatives.shape == (B, M, K, C)

    in_maps = []
    for b in range(B):
        idx = np.flatnonzero(mask[b])
        assert idx.size == M, f"row {b}: expected {M} masked, got {idx.size}"
        ctxg = np.ascontiguousarray(context[b].T[idx])  # [M, C]
        posg = np.ascontiguousarray(positive[b].T[idx])  # [M, C]
        in_maps.append(
            {
                "neg": np.ascontiguousarray(negatives[b]),
                "ctxg": ctxg,
                "posg": posg,
            }
        )

    res = run_bass_kernel_spmd(_get_nc(), in_maps, core_ids=list(range(B)))
    total = np.float64(0.0)
    for r in res.results:
        total += r["rowloss"].astype(np.float64).sum()
    return np.float32(total / (B * M))
